# revision 38
# baseline (speedup 1.0000x reference)
"""MoE gate routing kernel for Trainium2 (8 NeuronCores, Bass/Tile).

Computes, for hidden_states [4, 4096, 7168] (f32), gate kernel [7168, 256],
e_score_correction_bias [256]:
    logits = x @ W ; scores = sigmoid(logits) + bias
    grouped top-2-sum -> top-4 groups of 8 -> mask -> top-8 experts
    weights = 2.5 * topk_vals / sum(topk_vals)
Returns (topk_idx int32 [16384, 8], topk_weight f32 [16384, 8]).

Sharding: tokens split evenly across 8 cores (2048 tokens/core); W + bias
replicated. No cross-core communication.

Precision: the PE runs 16-bit matmuls at 1 cyc/row (4x the fp32 rate), so
logits are built from an exact 3-term split computed on the host:
    x = xh (fp16) + xl (bf16 residual)      [x to ~21 bits]
    w = wh (fp16) + wl (fp16 residual)      [w to ~19+ bits]
    logits = xh@wh + xh@wl + xl@bf16(w)
wl's values (<~8e-6) are fp16 subnormals; the PE honors them (verified on
HW), so all three terms accumulate into one PSUM tile at natural scale.
The dropped xl@wl term is O(2^-22) relative; resulting logit error ~6e-7
(validated on host and HW: 0/16384 topk mismatches vs the fp32 reference).
The x planes are shipped pre-transposed ([H, T]) so the PE spends no cycles
on transposes and the contraction dim lands on partitions directly.
"""
import sys

sys.path.insert(0, "/opt/trn_rl_repo")

import ml_dtypes
import numpy as np

import concourse.bass as bass  # noqa: F401
import concourse.mybir as mybir
import concourse.tile as tile
from concourse import bacc
from concourse.bass_utils import run_bass_kernel_spmd

# Problem constants (hardcoded per contract)
H = 7168
E = 256
N_CORES = 8
T_FULL = 4 * 4096           # 16384 tokens
T_C = T_FULL // N_CORES     # 2048 tokens per core
P = 128
KT = H // P                 # 56 contraction tiles
TT = T_C // P               # 16 token tiles per core
KB = 8                      # k-tiles per DMA block
NKB = KT // KB              # 7 k-blocks
# Phase 0 spans 5 token tiles so its PE work (~64us) covers the one-time W
# stream (~31us) on top of its own x stream; the 1-ttile final phase keeps
# the end-of-kernel topk tail short.
PHASES = [(0, 5), (5, 2), (7, 2), (9, 2), (11, 2), (13, 2), (15, 1)]
N_GROUP = 8
TOPK_GROUP = 4
TOP_K = 8
EPG = E // N_GROUP          # 32 experts per group
SCALE = 2.5

f32 = mybir.dt.float32
f16 = mybir.dt.float16
bf16 = mybir.dt.bfloat16
u32 = mybir.dt.uint32

_CACHED_NC = None


def _build_nc():
    nc = bacc.Bacc("TRN2", target_bir_lowering=False, debug=False)
    xh = nc.dram_tensor("xh", [H, T_C], f16, kind="ExternalInput")
    xl = nc.dram_tensor("xl", [H, T_C], bf16, kind="ExternalInput")
    wh = nc.dram_tensor("wh", [H, E], f16, kind="ExternalInput")
    wl = nc.dram_tensor("wl", [H, E], f16, kind="ExternalInput")
    wb = nc.dram_tensor("wb", [H, E], bf16, kind="ExternalInput")
    b = nc.dram_tensor("b", [E], f32, kind="ExternalInput")
    idx_out = nc.dram_tensor("idx_out", [T_C, TOP_K], u32, kind="ExternalOutput")
    wt_out = nc.dram_tensor("wt_out", [T_C, TOP_K], f32, kind="ExternalOutput")

    xh_r = xh.ap().rearrange("(ko p) t -> p ko t", p=P)  # [128, 56, 2048]
    xl_r = xl.ap().rearrange("(ko p) t -> p ko t", p=P)
    wh_r = wh.ap().rearrange("(ko p) e -> p ko e", p=P)  # [128, 56, 256]
    wl_r = wl.ap().rearrange("(ko p) e -> p ko e", p=P)
    wb_r = wb.ap().rearrange("(ko p) e -> p ko e", p=P)
    # token = t*128 + partition (natural order)
    idx_tp = idx_out.ap().rearrange("(t p) k -> p t k", p=P)
    wt_tp = wt_out.ap().rearrange("(t p) k -> p t k", p=P)

    with tile.TileContext(nc) as tc:
        with (
            tc.tile_pool(name="const", bufs=1) as cpool,
            tc.tile_pool(name="x0hp", bufs=3) as x0h_pool,
            tc.tile_pool(name="x0lp", bufs=3) as x0l_pool,
            tc.tile_pool(name="xhp", bufs=7) as xh_pool,
            tc.tile_pool(name="xlp", bufs=7) as xl_pool,
            tc.tile_pool(name="sc", bufs=2) as sc_pool,
            tc.tile_pool(name="tk", bufs=2) as tk_pool,
            tc.tile_pool(name="outp", bufs=1) as out_pool,
            tc.tile_pool(name="ps_a", bufs=6, space="PSUM") as ps_main,
        ):
            wh_sb = cpool.tile([P, KT, E], f16)
            wl_sb = cpool.tile([P, KT, E], f16)
            wb_sb = cpool.tile([P, KT, E], bf16)
            bias_sb = cpool.tile([P, E], f32)

            def wdma(sb, ap_, kb):
                sl = slice(kb * KB, (kb + 1) * KB)
                nc.sync.dma_start(sb[:, sl], ap_[:, sl])

            # W k-blocks are interleaved with phase-0's x stream below so the
            # first matmuls start as soon as block 0 of each plane lands.

            idx_acc = out_pool.tile([P, TT, TOP_K], u32)
            wt_acc = out_pool.tile([P, TT, TOP_K], f32)

            def topk_chain(T, p13):
                """sigmoid + grouped-top-k for one 128-token tile."""
                scores = sc_pool.tile([P, E], f32, tag="scores")
                nc.scalar.activation(
                    out=scores, in_=p13, func=mybir.ActivationFunctionType.Sigmoid
                )
                nc.vector.tensor_add(scores, scores, bias_sb)

                gmax8 = tk_pool.tile([P, N_GROUP, 8], f32, tag="gmax8")
                for g in range(N_GROUP):
                    nc.vector.max(out=gmax8[:, g], in_=scores[:, g * EPG : (g + 1) * EPG])
                gsum = tk_pool.tile([P, N_GROUP], f32, tag="gsum")
                nc.vector.tensor_add(gsum, gmax8[:, :, 0], gmax8[:, :, 1])
                gs8 = tk_pool.tile([P, 8], f32, tag="gs8")
                nc.vector.max(out=gs8, in_=gsum)
                gmask = tk_pool.tile([P, N_GROUP], f32, tag="gmask")
                nc.vector.tensor_scalar(
                    out=gmask, in0=gsum,
                    scalar1=gs8[:, TOPK_GROUP - 1 : TOPK_GROUP], scalar2=None,
                    op0=mybir.AluOpType.is_ge,
                )
                masked = sc_pool.tile([P, E], f32, tag="masked")
                nc.vector.tensor_mul(
                    masked.rearrange("p (g j) -> p g j", g=N_GROUP),
                    scores.rearrange("p (g j) -> p g j", g=N_GROUP),
                    gmask.unsqueeze(2).to_broadcast([P, N_GROUP, EPG]),
                )
                vals8 = tk_pool.tile([P, 8], f32, tag="vals8")
                nc.vector.max(out=vals8, in_=masked)
                nc.vector.max_index(out=idx_acc[:, T], in_max=vals8, in_values=masked)
                denom = tk_pool.tile([P, 1], f32, tag="denom")
                nc.vector.reduce_sum(out=denom, in_=vals8, axis=mybir.AxisListType.X)
                inv = tk_pool.tile([P, 1], f32, tag="inv")
                nc.vector.reciprocal(inv, denom)
                nc.vector.tensor_scalar(
                    out=wt_acc[:, T], in0=vals8,
                    scalar1=inv[:, 0:1], scalar2=SCALE,
                    op0=mybir.AluOpType.mult, op1=mybir.AluOpType.mult,
                )
                if T == TT - 2:
                    nc.sync.dma_start(idx_tp[:, : TT - 1], idx_acc[:, : TT - 1])
                    nc.sync.dma_start(wt_tp[:, : TT - 1], wt_acc[:, : TT - 1])
                elif T == TT - 1:
                    nc.sync.dma_start(idx_tp[:, TT - 1 :], idx_acc[:, TT - 1 :])
                    nc.sync.dma_start(wt_tp[:, TT - 1 :], wt_acc[:, TT - 1 :])

            for pi, (t0, nt) in enumerate(PHASES):
                tok = slice(t0 * P, (t0 + nt) * P)
                first, last = (pi == 0), (pi == len(PHASES) - 1)
                hp, lp = (x0h_pool, x0l_pool) if first else (xh_pool, xl_pool)
                # single accumulator per ttile: all three terms sum into it
                # (the wl plane is unscaled fp16 — subnormal values — so its
                # products land at the correct magnitude directly)
                p13 = [
                    ps_main.tile([P, E], f32, tag="ps", name=f"ps_{pi}_{i}")
                    for i in range(nt)
                ]
                for kb in range(NKB):
                    xh_t = hp.tile([P, KB, nt * P], f16, tag="xh", name=f"xh_{pi}_{kb}")
                    if first and kb == 0:
                        # split the very first loads so the first matmuls
                        # (ktl 0-3) start before the rest of the block lands
                        hf = KB // 2
                        nc.sync.dma_start(xh_t[:, :hf], xh_r[:, :hf, tok])
                        nc.sync.dma_start(wh_sb[:, :hf], wh_r[:, :hf])
                        nc.sync.dma_start(xh_t[:, hf:], xh_r[:, hf:KB, tok])
                        nc.sync.dma_start(wh_sb[:, hf:KB], wh_r[:, hf:KB])
                    else:
                        nc.sync.dma_start(xh_t, xh_r[:, kb * KB : (kb + 1) * KB, tok])
                        if first:
                            wdma(wh_sb, wh_r, kb)
                    xl_t = lp.tile([P, KB, nt * P], bf16, tag="xl", name=f"xl_{pi}_{kb}")
                    nc.sync.dma_start(xl_t, xl_r[:, kb * KB : (kb + 1) * KB, tok])
                    if first:
                        wdma(wl_sb, wl_r, kb)
                        wdma(wb_sb, wb_r, kb)
                        if kb == 0:
                            nc.sync.dma_start(
                                bias_sb, b.ap().unsqueeze(0).partition_broadcast(P)
                            )
                    def mm(term, ktl, tt):
                        kt = kb * KB + ktl
                        ts = slice(tt * P, (tt + 1) * P)
                        if term == 0:
                            nc.tensor.matmul(
                                p13[tt], xh_t[:, ktl, ts], wh_sb[:, kt],
                                start=(kt == 0), stop=False,
                            )
                        elif term == 1:
                            nc.tensor.matmul(
                                p13[tt], xh_t[:, ktl, ts], wl_sb[:, kt],
                                start=False, stop=False,
                            )
                        else:
                            nc.tensor.matmul(
                                p13[tt], xl_t[:, ktl, ts], wb_sb[:, kt],
                                start=False, stop=(kt == KT - 1),
                            )

                    if first and kb == 0:
                        # term-major: the xh@wh matmuls run while the xl/wl/wb
                        # DMAs are still streaming in
                        for term in range(3):
                            for ktl in range(KB):
                                for tt in range(nt):
                                    mm(term, ktl, tt)
                    elif last and kb == NKB - 1:
                        # finish ttile 0's accumulation first so its topk
                        # chain overlaps the remaining matmuls
                        for tt in range(nt):
                            for ktl in range(KB):
                                for term in range(3):
                                    mm(term, ktl, tt)
                    else:
                        for ktl in range(KB):
                            for tt in range(nt):
                                for term in range(3):
                                    mm(term, ktl, tt)
                    if last and kb == NKB - 1:
                        # chain for ttile 0 can start while ttile 1 finishes
                        topk_chain(t0, p13[0])
                if last:
                    for tt in range(1, nt):
                        topk_chain(t0 + tt, p13[tt])
                else:
                    for tt in range(nt):
                        topk_chain(t0 + tt, p13[tt])

    nc.compile()
    return nc


def get_nc():
    global _CACHED_NC
    if _CACHED_NC is None:
        _CACHED_NC = _build_nc()
    return _CACHED_NC


def _prep_planes(x_full, w_np):
    """Host-side split of x/w into the fp16/bf16 planes the kernel consumes."""
    bf = ml_dtypes.bfloat16
    xh_all = x_full.astype(np.float16)
    xl_all = (x_full - xh_all.astype(np.float32)).astype(bf)
    wh = np.ascontiguousarray(w_np.astype(np.float16))
    # fp16 subnormals: |w - wh| <= ~8e-6, quantum 2^-24 keeps ~7 bits
    wl = np.ascontiguousarray((w_np - wh.astype(np.float32)).astype(np.float16))
    wb = np.ascontiguousarray(w_np.astype(bf))
    return xh_all, xl_all, wh, wl, wb


def run(hidden_states, kernel_w, bias, trace=False, trace_cores=None):
    """Internal entry that also exposes trace results for benchmarking."""
    x_full = np.ascontiguousarray(
        np.asarray(hidden_states, dtype=np.float32).reshape(T_FULL, H)
    )
    w_np = np.ascontiguousarray(np.asarray(kernel_w, dtype=np.float32))
    b_np = np.ascontiguousarray(np.asarray(bias, dtype=np.float32))

    xh_all, xl_all, wh, wl, wb = _prep_planes(x_full, w_np)

    nc = get_nc()
    in_maps = []
    for c in range(N_CORES):
        rows = slice(c * T_C, (c + 1) * T_C)
        in_maps.append(
            {
                "xh": np.ascontiguousarray(xh_all[rows].T),
                "xl": np.ascontiguousarray(xl_all[rows].T),
                "wh": wh,
                "wl": wl,
                "wb": wb,
                "b": b_np,
            }
        )
    kw = {}
    if trace:
        kw = dict(trace=True, trace_cores=trace_cores or [0])
    last_err = None
    for attempt in range(3):
        try:
            res = run_bass_kernel_spmd(nc, in_maps, core_ids=list(range(N_CORES)), **kw)
            break
        except Exception as e:  # transient NRT/axon device hiccups
            last_err = e
            if attempt == 2:
                raise
            import time as _time

            _time.sleep(15)
    else:
        raise last_err

    idx = np.concatenate([r["idx_out"] for r in res.results], axis=0).astype(np.int32)
    wt = np.concatenate([r["wt_out"] for r in res.results], axis=0)
    return (idx, wt), res


def kernel(hidden_states, kernel, e_score_correction_bias):
    (idx, wt), _ = run(hidden_states, kernel, e_score_correction_bias)
    return idx, wt


# revision 55
# speedup vs baseline: 1.7439x; 1.7439x over previous
"""MoE gate routing kernel for Trainium2 (8 NeuronCores, Bass/Tile).

Computes, for hidden_states [4, 4096, 7168] (f32), gate kernel [7168, 256],
e_score_correction_bias [256]:
    logits = x @ W ; scores = sigmoid(logits) + bias
    grouped top-2-sum -> top-4 groups of 8 -> mask -> top-8 experts
    weights = 2.5 * topk_vals / sum(topk_vals)
Returns (topk_idx int32 [16384, 8], topk_weight f32 [16384, 8]).

Sharding: tokens split evenly across 8 cores (2048 tokens/core); W + bias
replicated. No cross-core communication.

Precision: logits are built from a 3-term split computed on the host:
    x = xh (fp16) + xl,   w = wh (fp16) + wl
    logits = xh@wh  +  2^-15 * ( e4m3(xh) @ e4m3(wl*2^15)
                               + e4m3(xl*2^11) @ e4m3(w*2^4) )
The main term runs as fp16 matmuls (1 cyc/row); both correction terms are
~2^-11 relative and only need a few bits, so they run as fp8e4m3 matmuls in
DoubleRow mode (0.5 cyc/row, two k-tiles per instruction) and share one
scaled PSUM accumulator (both products carry the same 2^15 factor).
Resulting logit error ~1e-5 -> topk weights match the fp32 reference to
~5e-6 relative on the reference inputs.
The x planes are shipped pre-transposed ([H, T]) so the PE spends no cycles
on transposes; fp8 planes are additionally pair/phase-blocked on the host so
every DMA descriptor stays >= 512B contiguous.
"""
import sys

sys.path.insert(0, "/opt/trn_rl_repo")

import ml_dtypes
import numpy as np

import concourse.bass as bass  # noqa: F401
import concourse.mybir as mybir
import concourse.tile as tile
from concourse import bacc
from concourse.bass_utils import run_bass_kernel_spmd

# Problem constants (hardcoded per contract)
H = 7168
E = 256
N_CORES = 8
T_FULL = 4 * 4096           # 16384 tokens
T_C = T_FULL // N_CORES     # 2048 tokens per core
P = 128
KT = H // P                 # 56 contraction tiles
KP = KT // 2                # 28 k-tile pairs (DoubleRow granularity)
TT = T_C // P               # 16 token tiles per core
KB = 8                      # k-tiles per DMA block
KPB = KB // 2               # 4 k-pairs per DMA block
NKB = KT // KB              # 7 k-blocks
PH = 8                      # phases of 2 token tiles
TPP = 2
N_GROUP = 8
TOPK_GROUP = 4
TOP_K = 8
EPG = E // N_GROUP          # 32 experts per group
SCALE = 2.5
XL_S = 2.0 ** 11            # xl plane pre-scale (keeps e4m3 normal)
WL_S = 2.0 ** 15            # wl plane pre-scale
WH8_S = 2.0 ** 4            # w plane pre-scale for the xl correction
PS2_S = 2.0 ** -15          # shared descale of the correction PSUM

f32 = mybir.dt.float32
f16 = mybir.dt.float16
f8 = mybir.dt.float8e4
u32 = mybir.dt.uint32

_CACHED_NC = None


def _build_nc():
    nc = bacc.Bacc("TRN2", target_bir_lowering=False, debug=False)
    xh = nc.dram_tensor("xh", [H, T_C], f16, kind="ExternalInput")
    # fp8 plane arrives pair/phase-blocked: [kp, p, phase, two, tok]
    xl8 = nc.dram_tensor("xl8", [KP, P, PH, 2, TPP * P], f8, kind="ExternalInput")
    wh = nc.dram_tensor("wh", [H, E], f16, kind="ExternalInput")
    # fp8 W planes pair-blocked: [kp, p, two, e]
    wl8 = nc.dram_tensor("wl8", [KP, P, 2, E], f8, kind="ExternalInput")
    wh8 = nc.dram_tensor("wh8", [KP, P, 2, E], f8, kind="ExternalInput")
    b = nc.dram_tensor("b", [E], f32, kind="ExternalInput")
    idx_out = nc.dram_tensor("idx_out", [T_C, TOP_K], u32, kind="ExternalOutput")
    wt_out = nc.dram_tensor("wt_out", [T_C, TOP_K], f32, kind="ExternalOutput")

    xh_r = xh.ap().rearrange("(ko p) t -> p ko t", p=P)      # [128, 56, 2048]
    wh_r = wh.ap().rearrange("(ko p) e -> p ko e", p=P)      # [128, 56, 256]
    xl8_r = xl8.ap().rearrange("kp p ph two t -> p kp ph two t")
    wl8_r = wl8.ap().rearrange("kp p two e -> p kp two e")   # [128, 28, 2, 256]
    wh8_r = wh8.ap().rearrange("kp p two e -> p kp two e")
    # token = t*128 + partition (natural order)
    idx_tp = idx_out.ap().rearrange("(t p) k -> p t k", p=P)
    wt_tp = wt_out.ap().rearrange("(t p) k -> p t k", p=P)

    DR = mybir.MatmulPerfMode.DoubleRow

    with tile.TileContext(nc) as tc:
        with (
            tc.tile_pool(name="const", bufs=1) as cpool,
            tc.tile_pool(name="xhp", bufs=9) as xh_pool,
            tc.tile_pool(name="x8p", bufs=9) as x8_pool,
            tc.tile_pool(name="xl8p", bufs=9) as xl8_pool,
            tc.tile_pool(name="sc", bufs=2) as sc_pool,
            tc.tile_pool(name="tk", bufs=2) as tk_pool,
            tc.tile_pool(name="outp", bufs=1) as out_pool,
            tc.tile_pool(name="ps_a", bufs=4, space="PSUM") as ps_main,
            tc.tile_pool(name="ps_b", bufs=4, space="PSUM") as ps_cor,
        ):
            wh_sb = cpool.tile([P, KT, E], f16)
            wl8_sb = cpool.tile([P, KP, 2, E], f8)
            wh8_sb = cpool.tile([P, KP, 2, E], f8)
            bias_sb = cpool.tile([P, E], f32)

            idx_acc = out_pool.tile([P, TT, TOP_K], u32)
            wt_acc = out_pool.tile([P, TT, TOP_K], f32)

            def topk_chain(T, p1, p2):
                """descale+combine, sigmoid, grouped-top-k for one 128-token tile."""
                comb = sc_pool.tile([P, E], f32, tag="comb")
                nc.vector.tensor_scalar(
                    out=comb, in0=p2, scalar1=PS2_S, scalar2=None,
                    op0=mybir.AluOpType.mult,
                )
                nc.vector.tensor_add(comb, comb, p1)
                scores = sc_pool.tile([P, E], f32, tag="scores")
                nc.scalar.activation(
                    out=scores, in_=comb, func=mybir.ActivationFunctionType.Sigmoid
                )
                nc.vector.tensor_add(scores, scores, bias_sb)

                gmax8 = tk_pool.tile([P, N_GROUP, 8], f32, tag="gmax8")
                for g in range(N_GROUP):
                    nc.vector.max(out=gmax8[:, g], in_=scores[:, g * EPG : (g + 1) * EPG])
                gsum = tk_pool.tile([P, N_GROUP], f32, tag="gsum")
                nc.vector.tensor_add(gsum, gmax8[:, :, 0], gmax8[:, :, 1])
                gs8 = tk_pool.tile([P, 8], f32, tag="gs8")
                nc.vector.max(out=gs8, in_=gsum)
                gmask = tk_pool.tile([P, N_GROUP], f32, tag="gmask")
                nc.vector.tensor_scalar(
                    out=gmask, in0=gsum,
                    scalar1=gs8[:, TOPK_GROUP - 1 : TOPK_GROUP], scalar2=None,
                    op0=mybir.AluOpType.is_ge,
                )
                masked = sc_pool.tile([P, E], f32, tag="masked")
                nc.vector.tensor_mul(
                    masked.rearrange("p (g j) -> p g j", g=N_GROUP),
                    scores.rearrange("p (g j) -> p g j", g=N_GROUP),
                    gmask.unsqueeze(2).to_broadcast([P, N_GROUP, EPG]),
                )
                vals8 = tk_pool.tile([P, 8], f32, tag="vals8")
                nc.vector.max(out=vals8, in_=masked)
                nc.vector.max_index(out=idx_acc[:, T], in_max=vals8, in_values=masked)
                if T == TT - 1:
                    # idx leaves while the weight math still runs
                    nc.sync.dma_start(idx_tp[:, TT - 1 :], idx_acc[:, TT - 1 :])
                denom = tk_pool.tile([P, 1], f32, tag="denom")
                nc.vector.reduce_sum(out=denom, in_=vals8, axis=mybir.AxisListType.X)
                inv = tk_pool.tile([P, 1], f32, tag="inv")
                nc.vector.reciprocal(inv, denom)
                nc.vector.tensor_scalar(
                    out=wt_acc[:, T], in0=vals8,
                    scalar1=inv[:, 0:1], scalar2=SCALE,
                    op0=mybir.AluOpType.mult, op1=mybir.AluOpType.mult,
                )
                if T == TT - 2:
                    nc.sync.dma_start(idx_tp[:, : TT - 1], idx_acc[:, : TT - 1])
                    nc.sync.dma_start(wt_tp[:, : TT - 1], wt_acc[:, : TT - 1])
                elif T == TT - 1:
                    nc.sync.dma_start(wt_tp[:, TT - 1 :], wt_acc[:, TT - 1 :])

            xh_tiles = {}

            def issue_xh(pi2, kb2):
                tok2 = slice(pi2 * TPP * P, (pi2 + 1) * TPP * P)
                t = xh_pool.tile(
                    [P, KB, TPP * P], f16, tag="xh", name=f"xh_{pi2}_{kb2}"
                )
                if pi2 == 0 and kb2 == 0:
                    hf = KB // 2
                    nc.sync.dma_start(t[:, :hf], xh_r[:, :hf, tok2])
                    nc.sync.dma_start(wh_sb[:, :hf], wh_r[:, :hf])
                    nc.sync.dma_start(t[:, hf:], xh_r[:, hf:KB, tok2])
                    nc.sync.dma_start(wh_sb[:, hf:KB], wh_r[:, hf:KB])
                else:
                    ks2 = slice(kb2 * KB, (kb2 + 1) * KB)
                    nc.sync.dma_start(t, xh_r[:, ks2, tok2])
                xh_tiles[(pi2, kb2)] = t

            issue_xh(0, 0)
            for pi in range(PH):
                tok = slice(pi * TPP * P, (pi + 1) * TPP * P)
                first, last = (pi == 0), (pi == PH - 1)
                p1 = [
                    ps_main.tile([P, E], f32, tag="p1", name=f"p1_{pi}_{i}")
                    for i in range(TPP)
                ]
                p2 = [
                    ps_cor.tile([P, E], f32, tag="p2", name=f"p2_{pi}_{i}")
                    for i in range(TPP)
                ]
                for kb in range(NKB):
                    ks = slice(kb * KB, (kb + 1) * KB)
                    kps = slice(kb * KPB, (kb + 1) * KPB)
                    xh_t = xh_tiles.pop((pi, kb))
                    # prefetch the NEXT k-block's xh one step ahead so its
                    # Act fp16->fp8 conversion is off the PE critical path
                    nxt = (pi, kb + 1) if kb + 1 < NKB else (pi + 1, 0)
                    if nxt[0] < PH:
                        issue_xh(*nxt)
                    if first and kb != 0:
                        nc.sync.dma_start(wh_sb[:, ks], wh_r[:, ks])
                    # x8 is derived on-chip: fp16 -> e4m3 copy on the
                    # (otherwise idle) scalar engine saves a 40us DMA plane
                    x8_t = x8_pool.tile([P, KPB, 2, TPP * P], f8, tag="x8", name=f"x8_{pi}_{kb}")
                    nc.scalar.copy(
                        out=x8_t.rearrange("p kp two t -> p (kp two) t"),
                        in_=xh_t,
                    )
                    if first:
                        nc.sync.dma_start(wl8_sb[:, kps], wl8_r[:, kps])
                    xl8_t = xl8_pool.tile([P, KPB, 2, TPP * P], f8, tag="xl8", name=f"xl8_{pi}_{kb}")
                    nc.sync.dma_start(xl8_t, xl8_r[:, kps, pi])
                    if first:
                        nc.sync.dma_start(wh8_sb[:, kps], wh8_r[:, kps])
                        if kb == 0:
                            nc.sync.dma_start(
                                bias_sb, b.ap().unsqueeze(0).partition_broadcast(P)
                            )

                    def t1(ktl, tt):
                        kt = kb * KB + ktl
                        ts = slice(tt * P, (tt + 1) * P)
                        nc.tensor.matmul(
                            p1[tt], xh_t[:, ktl, ts], wh_sb[:, kt],
                            start=(kt == 0), stop=(kt == KT - 1),
                        )

                    def t23(term, kpl, tt):
                        kp = kb * KPB + kpl
                        ts = slice(tt * P, (tt + 1) * P)
                        if term == 0:
                            nc.tensor.matmul(
                                p2[tt], x8_t[:, kpl, :, ts], wl8_sb[:, kp],
                                start=(kp == 0), stop=False, perf_mode=DR,
                            )
                        else:
                            nc.tensor.matmul(
                                p2[tt], xl8_t[:, kpl, :, ts], wh8_sb[:, kp],
                                start=False, stop=(kp == KP - 1), perf_mode=DR,
                            )

                    if last and kb == NKB - 1:
                        # finish ttile 0 first so its chain overlaps the rest
                        for tt in range(TPP):
                            for ktl in range(KB):
                                t1(ktl, tt)
                            for kpl in range(KPB):
                                t23(0, kpl, tt)
                                t23(1, kpl, tt)
                            if tt == 0:
                                topk_chain(pi * TPP, p1[0], p2[0])
                    else:
                        for ktl in range(KB):
                            for tt in range(TPP):
                                t1(ktl, tt)
                        for kpl in range(KPB):
                            for tt in range(TPP):
                                t23(0, kpl, tt)
                        for kpl in range(KPB):
                            for tt in range(TPP):
                                t23(1, kpl, tt)
                if last:
                    topk_chain(pi * TPP + 1, p1[1], p2[1])
                else:
                    for tt in range(TPP):
                        topk_chain(pi * TPP + tt, p1[tt], p2[tt])

    nc.compile()
    return nc


def get_nc():
    global _CACHED_NC
    if _CACHED_NC is None:
        _CACHED_NC = _build_nc()
    return _CACHED_NC


def _prep_planes(x_full, w_np):
    """Host-side split of x/w into the fp16/fp8 planes the kernel consumes."""
    e4 = ml_dtypes.float8_e4m3
    xh_all = x_full.astype(np.float16)               # [T, H]
    xl_all = x_full - xh_all.astype(np.float32)      # f32 residual
    wh = np.ascontiguousarray(w_np.astype(np.float16))
    wh32 = wh.astype(np.float32)
    wl8 = ((w_np - wh32) * WL_S).astype(e4)          # [H, E]
    wh8 = (w_np * WH8_S).astype(e4)
    return xh_all, xl_all, wh, wl8, wh8


def _block_x(plane_t):
    """[H, T_C] -> [KP, P, PH, 2, 256]: pair/phase-blocked for 512B descs."""
    return np.ascontiguousarray(
        plane_t.reshape(KP, 2, P, PH, TPP * P).transpose(0, 2, 3, 1, 4)
    )


def _block_w(plane):
    """[H, E] -> [KP, P, 2, E]."""
    return np.ascontiguousarray(plane.reshape(KP, 2, P, E).transpose(0, 2, 1, 3))


def run(hidden_states, kernel_w, bias, trace=False, trace_cores=None):
    """Internal entry that also exposes trace results for benchmarking."""
    e4 = ml_dtypes.float8_e4m3
    x_full = np.ascontiguousarray(
        np.asarray(hidden_states, dtype=np.float32).reshape(T_FULL, H)
    )
    w_np = np.ascontiguousarray(np.asarray(kernel_w, dtype=np.float32))
    b_np = np.ascontiguousarray(np.asarray(bias, dtype=np.float32))

    xh_all, xl_all, wh, wl8, wh8 = _prep_planes(x_full, w_np)
    wl8_b, wh8_b = _block_w(wl8), _block_w(wh8)

    nc = get_nc()
    in_maps = []
    for c in range(N_CORES):
        rows = slice(c * T_C, (c + 1) * T_C)
        xh_t = np.ascontiguousarray(xh_all[rows].T)          # [H, T_C] fp16
        xl_t = xl_all[rows].T                                # [H, T_C] f32 view
        in_maps.append(
            {
                "xh": xh_t,
                "xl8": _block_x((xl_t * XL_S).astype(e4)),
                "wh": wh,
                "wl8": wl8_b,
                "wh8": wh8_b,
                "b": b_np,
            }
        )
    kw = {}
    if trace:
        kw = dict(trace=True, trace_cores=trace_cores or [0])
    last_err = None
    for attempt in range(3):
        try:
            res = run_bass_kernel_spmd(nc, in_maps, core_ids=list(range(N_CORES)), **kw)
            break
        except Exception as e:  # transient NRT/axon device hiccups
            last_err = e
            if attempt == 2:
                raise
            import time as _time

            _time.sleep(15)
    else:
        raise last_err

    idx = np.concatenate([r["idx_out"] for r in res.results], axis=0).astype(np.int32)
    wt = np.concatenate([r["wt_out"] for r in res.results], axis=0)
    return (idx, wt), res


def kernel(hidden_states, kernel, e_score_correction_bias):
    (idx, wt), _ = run(hidden_states, kernel, e_score_correction_bias)
    return idx, wt


# revision 61
# speedup vs baseline: 1.7575x; 1.0078x over previous
"""MoE gate routing kernel for Trainium2 (8 NeuronCores, Bass/Tile).

Computes, for hidden_states [4, 4096, 7168] (f32), gate kernel [7168, 256],
e_score_correction_bias [256]:
    logits = x @ W ; scores = sigmoid(logits) + bias
    grouped top-2-sum -> top-4 groups of 8 -> mask -> top-8 experts
    weights = 2.5 * topk_vals / sum(topk_vals)
Returns (topk_idx int32 [16384, 8], topk_weight f32 [16384, 8]).

Sharding: tokens split evenly across 8 cores (2048 tokens/core); W + bias
replicated. No cross-core communication.

Precision: logits are built from a 3-term split computed on the host:
    x = xh (fp16) + xl,   w = wh (fp16) + wl
    logits = xh@wh  +  2^-15 * ( e4m3(xh) @ e4m3(wl*2^15)
                               + e4m3(xl*2^11) @ e4m3(w*2^4) )
The main term runs as fp16 matmuls (1 cyc/row); both correction terms are
~2^-11 relative and only need a few bits, so they run as fp8e4m3 matmuls in
DoubleRow mode (0.5 cyc/row, two k-tiles per instruction) and share one
scaled PSUM accumulator (both products carry the same 2^15 factor).
Resulting logit error ~1e-5 -> topk weights match the fp32 reference to
~5e-6 relative on the reference inputs.
The x planes are shipped pre-transposed ([H, T]) so the PE spends no cycles
on transposes; fp8 planes are additionally pair/phase-blocked on the host so
every DMA descriptor stays >= 512B contiguous.
"""
import sys

sys.path.insert(0, "/opt/trn_rl_repo")

import ml_dtypes
import numpy as np

import concourse.bass as bass  # noqa: F401
import concourse.mybir as mybir
import concourse.tile as tile
from concourse import bacc
from concourse.bass_utils import run_bass_kernel_spmd

# Problem constants (hardcoded per contract)
H = 7168
E = 256
N_CORES = 8
T_FULL = 4 * 4096           # 16384 tokens
T_C = T_FULL // N_CORES     # 2048 tokens per core
P = 128
KT = H // P                 # 56 contraction tiles
KP = KT // 2                # 28 k-tile pairs (DoubleRow granularity)
TT = T_C // P               # 16 token tiles per core
KB = 8                      # k-tiles per DMA block
KPB = KB // 2               # 4 k-pairs per DMA block
NKB = KT // KB              # 7 k-blocks
PH = 8                      # phases of 2 token tiles
TPP = 2
N_GROUP = 8
TOPK_GROUP = 4
TOP_K = 8
EPG = E // N_GROUP          # 32 experts per group
SCALE = 2.5
XL_S = 2.0 ** 11            # xl plane pre-scale (keeps e4m3 normal)
WL_S = 2.0 ** 15            # wl plane pre-scale
WH8_S = 2.0 ** 4            # w plane pre-scale for the xl correction
PS2_S = 2.0 ** -15          # shared descale of the correction PSUM

f32 = mybir.dt.float32
f16 = mybir.dt.float16
f8 = mybir.dt.float8e4
u32 = mybir.dt.uint32

_CACHED_NC = None


def _build_nc():
    nc = bacc.Bacc("TRN2", target_bir_lowering=False, debug=False)
    xh = nc.dram_tensor("xh", [H, T_C], f16, kind="ExternalInput")
    # fp8 plane arrives pair/phase-blocked: [kp, p, phase, two, tok]
    xl8 = nc.dram_tensor("xl8", [KP, P, PH, 2, TPP * P], f8, kind="ExternalInput")
    wh = nc.dram_tensor("wh", [H, E], f16, kind="ExternalInput")
    # fp8 W planes pair-blocked: [kp, p, two, e]
    wl8 = nc.dram_tensor("wl8", [KP, P, 2, E], f8, kind="ExternalInput")
    wh8 = nc.dram_tensor("wh8", [KP, P, 2, E], f8, kind="ExternalInput")
    b = nc.dram_tensor("b", [E], f32, kind="ExternalInput")
    idx_out = nc.dram_tensor("idx_out", [T_C, TOP_K], u32, kind="ExternalOutput")
    wt_out = nc.dram_tensor("wt_out", [T_C, TOP_K], f32, kind="ExternalOutput")

    xh_r = xh.ap().rearrange("(ko p) t -> p ko t", p=P)      # [128, 56, 2048]
    wh_r = wh.ap().rearrange("(ko p) e -> p ko e", p=P)      # [128, 56, 256]
    xl8_r = xl8.ap().rearrange("kp p ph two t -> p kp ph two t")
    wl8_r = wl8.ap().rearrange("kp p two e -> p kp two e")   # [128, 28, 2, 256]
    wh8_r = wh8.ap().rearrange("kp p two e -> p kp two e")
    # token = t*128 + partition (natural order)
    idx_tp = idx_out.ap().rearrange("(t p) k -> p t k", p=P)
    wt_tp = wt_out.ap().rearrange("(t p) k -> p t k", p=P)

    DR = mybir.MatmulPerfMode.DoubleRow

    with tile.TileContext(nc) as tc:
        with (
            tc.tile_pool(name="const", bufs=1) as cpool,
            tc.tile_pool(name="xhp", bufs=9) as xh_pool,
            tc.tile_pool(name="x8p", bufs=9) as x8_pool,
            tc.tile_pool(name="xl8p", bufs=9) as xl8_pool,
            tc.tile_pool(name="sc", bufs=2) as sc_pool,
            tc.tile_pool(name="tk", bufs=2) as tk_pool,
            tc.tile_pool(name="outp", bufs=1) as out_pool,
            tc.tile_pool(name="ps_a", bufs=4, space="PSUM") as ps_main,
            tc.tile_pool(name="ps_b", bufs=4, space="PSUM") as ps_cor,
        ):
            wh_sb = cpool.tile([P, KT, E], f16)
            wl8_sb = cpool.tile([P, KP, 2, E], f8)
            wh8_sb = cpool.tile([P, KP, 2, E], f8)
            bias_sb = cpool.tile([P, E], f32)

            idx_acc = out_pool.tile([P, TT, TOP_K], u32)
            wt_acc = out_pool.tile([P, TT, TOP_K], f32)

            def topk_chain(T, p1, p2):
                """descale+combine, sigmoid, grouped-top-k for one 128-token tile."""
                comb = sc_pool.tile([P, E], f32, tag="comb")
                nc.vector.tensor_scalar(
                    out=comb, in0=p2, scalar1=PS2_S, scalar2=None,
                    op0=mybir.AluOpType.mult,
                )
                nc.vector.tensor_add(comb, comb, p1)
                scores = sc_pool.tile([P, E], f32, tag="scores")
                nc.scalar.activation(
                    out=scores, in_=comb, func=mybir.ActivationFunctionType.Sigmoid
                )
                nc.vector.tensor_add(scores, scores, bias_sb)

                gmax8 = tk_pool.tile([P, N_GROUP, 8], f32, tag="gmax8")
                for g in range(N_GROUP):
                    nc.vector.max(out=gmax8[:, g], in_=scores[:, g * EPG : (g + 1) * EPG])
                gsum = tk_pool.tile([P, N_GROUP], f32, tag="gsum")
                nc.vector.tensor_add(gsum, gmax8[:, :, 0], gmax8[:, :, 1])
                gs8 = tk_pool.tile([P, 8], f32, tag="gs8")
                nc.vector.max(out=gs8, in_=gsum)
                gmask = tk_pool.tile([P, N_GROUP], f32, tag="gmask")
                nc.vector.tensor_scalar(
                    out=gmask, in0=gsum,
                    scalar1=gs8[:, TOPK_GROUP - 1 : TOPK_GROUP], scalar2=None,
                    op0=mybir.AluOpType.is_ge,
                )
                masked = sc_pool.tile([P, E], f32, tag="masked")
                nc.vector.tensor_mul(
                    masked.rearrange("p (g j) -> p g j", g=N_GROUP),
                    scores.rearrange("p (g j) -> p g j", g=N_GROUP),
                    gmask.unsqueeze(2).to_broadcast([P, N_GROUP, EPG]),
                )
                vals8 = tk_pool.tile([P, 8], f32, tag="vals8")
                nc.vector.max(out=vals8, in_=masked)
                nc.vector.max_index(out=idx_acc[:, T], in_max=vals8, in_values=masked)
                if T == TT - 1:
                    # idx leaves while the weight math still runs
                    nc.sync.dma_start(idx_tp[:, TT - 1 :], idx_acc[:, TT - 1 :])
                denom = tk_pool.tile([P, 1], f32, tag="denom")
                nc.vector.reduce_sum(out=denom, in_=vals8, axis=mybir.AxisListType.X)
                inv = tk_pool.tile([P, 1], f32, tag="inv")
                nc.vector.reciprocal(inv, denom)
                nc.vector.tensor_scalar(
                    out=wt_acc[:, T], in0=vals8,
                    scalar1=inv[:, 0:1], scalar2=SCALE,
                    op0=mybir.AluOpType.mult, op1=mybir.AluOpType.mult,
                )
                if T == TT - 3:
                    # bulk leaves while the last phase computes
                    nc.sync.dma_start(idx_tp[:, : TT - 2], idx_acc[:, : TT - 2])
                    nc.sync.dma_start(wt_tp[:, : TT - 2], wt_acc[:, : TT - 2])
                elif T == TT - 2:
                    nc.sync.dma_start(idx_tp[:, T : T + 1], idx_acc[:, T : T + 1])
                    nc.sync.dma_start(wt_tp[:, T : T + 1], wt_acc[:, T : T + 1])
                elif T == TT - 1:
                    nc.sync.dma_start(wt_tp[:, TT - 1 :], wt_acc[:, TT - 1 :])

            xh_tiles = {}

            def issue_xh(pi2, kb2):
                tok2 = slice(pi2 * TPP * P, (pi2 + 1) * TPP * P)
                t = xh_pool.tile(
                    [P, KB, TPP * P], f16, tag="xh", name=f"xh_{pi2}_{kb2}"
                )
                if pi2 == 0 and kb2 == 0:
                    hf = KB // 2
                    nc.sync.dma_start(t[:, :hf], xh_r[:, :hf, tok2])
                    nc.sync.dma_start(wh_sb[:, :hf], wh_r[:, :hf])
                    nc.sync.dma_start(t[:, hf:], xh_r[:, hf:KB, tok2])
                    nc.sync.dma_start(wh_sb[:, hf:KB], wh_r[:, hf:KB])
                else:
                    ks2 = slice(kb2 * KB, (kb2 + 1) * KB)
                    nc.sync.dma_start(t, xh_r[:, ks2, tok2])
                xh_tiles[(pi2, kb2)] = t

            issue_xh(0, 0)
            for pi in range(PH):
                tok = slice(pi * TPP * P, (pi + 1) * TPP * P)
                first, last = (pi == 0), (pi == PH - 1)
                p1 = [
                    ps_main.tile([P, E], f32, tag="p1", name=f"p1_{pi}_{i}")
                    for i in range(TPP)
                ]
                p2 = [
                    ps_cor.tile([P, E], f32, tag="p2", name=f"p2_{pi}_{i}")
                    for i in range(TPP)
                ]
                mm_blocks = []

                def t1(kb, xh_t, ktl, tt):
                    kt = kb * KB + ktl
                    ts = slice(tt * P, (tt + 1) * P)
                    nc.tensor.matmul(
                        p1[tt], xh_t[:, ktl, ts], wh_sb[:, kt],
                        start=(kt == 0), stop=(kt == KT - 1),
                    )

                def t23(kb, x8_t, xl8_t, term, kpl, tt):
                    kp = kb * KPB + kpl
                    ts = slice(tt * P, (tt + 1) * P)
                    if term == 0:
                        nc.tensor.matmul(
                            p2[tt], x8_t[:, kpl, :, ts], wl8_sb[:, kp],
                            start=(kp == 0), stop=False, perf_mode=DR,
                        )
                    else:
                        nc.tensor.matmul(
                            p2[tt], xl8_t[:, kpl, :, ts], wh8_sb[:, kp],
                            start=False, stop=(kp == KP - 1), perf_mode=DR,
                        )

                for kb in range(NKB):
                    ks = slice(kb * KB, (kb + 1) * KB)
                    kps = slice(kb * KPB, (kb + 1) * KPB)
                    xh_t = xh_tiles.pop((pi, kb))
                    # prefetch the NEXT k-block's xh one step ahead so its
                    # Act fp16->fp8 conversion is off the PE critical path
                    nxt = (pi, kb + 1) if kb + 1 < NKB else (pi + 1, 0)
                    if nxt[0] < PH:
                        issue_xh(*nxt)
                    if first and kb != 0:
                        nc.sync.dma_start(wh_sb[:, ks], wh_r[:, ks])
                    # x8 is derived on-chip: fp16 -> e4m3 copy on the
                    # (otherwise idle) scalar engine saves a 40us DMA plane
                    x8_t = x8_pool.tile([P, KPB, 2, TPP * P], f8, tag="x8", name=f"x8_{pi}_{kb}")
                    nc.scalar.copy(
                        out=x8_t.rearrange("p kp two t -> p (kp two) t"),
                        in_=xh_t,
                    )
                    if first:
                        nc.sync.dma_start(wl8_sb[:, kps], wl8_r[:, kps])
                    xl8_t = xl8_pool.tile([P, KPB, 2, TPP * P], f8, tag="xl8", name=f"xl8_{pi}_{kb}")
                    nc.sync.dma_start(xl8_t, xl8_r[:, kps, pi])
                    if first:
                        nc.sync.dma_start(wh8_sb[:, kps], wh8_r[:, kps])
                        if kb == 0:
                            nc.sync.dma_start(
                                bias_sb, b.ap().unsqueeze(0).partition_broadcast(P)
                            )

                    if last:
                        # defer matmuls: emitted ttile-serial after all DMAs
                        mm_blocks.append((kb, xh_t, x8_t, xl8_t))
                    else:
                        for ktl in range(KB):
                            for tt in range(TPP):
                                t1(kb, xh_t, ktl, tt)
                        for kpl in range(KPB):
                            for tt in range(TPP):
                                t23(kb, x8_t, xl8_t, 0, kpl, tt)
                        for kpl in range(KPB):
                            for tt in range(TPP):
                                t23(kb, x8_t, xl8_t, 1, kpl, tt)
                if last:
                    # full k-range for ttile 0, its chain, then ttile 1
                    for tt in range(TPP):
                        for kb2, xh_t2, x8_t2, xl8_t2 in mm_blocks:
                            for ktl in range(KB):
                                t1(kb2, xh_t2, ktl, tt)
                            for kpl in range(KPB):
                                t23(kb2, x8_t2, xl8_t2, 0, kpl, tt)
                                t23(kb2, x8_t2, xl8_t2, 1, kpl, tt)
                        topk_chain(pi * TPP + tt, p1[tt], p2[tt])
                else:
                    for tt in range(TPP):
                        topk_chain(pi * TPP + tt, p1[tt], p2[tt])

    nc.compile()
    return nc


def get_nc():
    global _CACHED_NC
    if _CACHED_NC is None:
        _CACHED_NC = _build_nc()
    return _CACHED_NC


def _prep_planes(x_full, w_np):
    """Host-side split of x/w into the fp16/fp8 planes the kernel consumes."""
    e4 = ml_dtypes.float8_e4m3
    xh_all = x_full.astype(np.float16)               # [T, H]
    xl_all = x_full - xh_all.astype(np.float32)      # f32 residual
    wh = np.ascontiguousarray(w_np.astype(np.float16))
    wh32 = wh.astype(np.float32)
    wl8 = ((w_np - wh32) * WL_S).astype(e4)          # [H, E]
    wh8 = (w_np * WH8_S).astype(e4)
    return xh_all, xl_all, wh, wl8, wh8


def _block_x(plane_t):
    """[H, T_C] -> [KP, P, PH, 2, 256]: pair/phase-blocked for 512B descs."""
    return np.ascontiguousarray(
        plane_t.reshape(KP, 2, P, PH, TPP * P).transpose(0, 2, 3, 1, 4)
    )


def _block_w(plane):
    """[H, E] -> [KP, P, 2, E]."""
    return np.ascontiguousarray(plane.reshape(KP, 2, P, E).transpose(0, 2, 1, 3))


def run(hidden_states, kernel_w, bias, trace=False, trace_cores=None):
    """Internal entry that also exposes trace results for benchmarking."""
    e4 = ml_dtypes.float8_e4m3
    x_full = np.ascontiguousarray(
        np.asarray(hidden_states, dtype=np.float32).reshape(T_FULL, H)
    )
    w_np = np.ascontiguousarray(np.asarray(kernel_w, dtype=np.float32))
    b_np = np.ascontiguousarray(np.asarray(bias, dtype=np.float32))

    xh_all, xl_all, wh, wl8, wh8 = _prep_planes(x_full, w_np)
    wl8_b, wh8_b = _block_w(wl8), _block_w(wh8)

    nc = get_nc()
    in_maps = []
    for c in range(N_CORES):
        rows = slice(c * T_C, (c + 1) * T_C)
        xh_t = np.ascontiguousarray(xh_all[rows].T)          # [H, T_C] fp16
        xl_t = xl_all[rows].T                                # [H, T_C] f32 view
        in_maps.append(
            {
                "xh": xh_t,
                "xl8": _block_x((xl_t * XL_S).astype(e4)),
                "wh": wh,
                "wl8": wl8_b,
                "wh8": wh8_b,
                "b": b_np,
            }
        )
    kw = {}
    if trace:
        kw = dict(trace=True, trace_cores=trace_cores or [0])
    last_err = None
    for attempt in range(3):
        try:
            res = run_bass_kernel_spmd(nc, in_maps, core_ids=list(range(N_CORES)), **kw)
            break
        except Exception as e:  # transient NRT/axon device hiccups
            last_err = e
            if attempt == 2:
                raise
            import time as _time

            _time.sleep(15)
    else:
        raise last_err

    idx = np.concatenate([r["idx_out"] for r in res.results], axis=0).astype(np.int32)
    wt = np.concatenate([r["wt_out"] for r in res.results], axis=0)
    return (idx, wt), res


def kernel(hidden_states, kernel, e_score_correction_bias):
    (idx, wt), _ = run(hidden_states, kernel, e_score_correction_bias)
    return idx, wt
